# revision 12
# baseline (speedup 1.0000x reference)
"""Trainium2 Bass kernel: silhouette rasterizer (nn_NeuralRenderer).

Algorithm
---------
Reference renders a silhouette by testing every pixel against every face's
three edge functions.  We instead exploit that each edge function is affine
in pixel coords: for a fixed image row y the inside-test reduces to an
x-interval [lo, hi] per (face, row).  The silhouette row is the union of
the per-face intervals.

Host (numpy, tiny): project vertices exactly as the reference, gather per
face, build sign-normalized edge coefficients A,B,C per edge (w = A*px +
B*py + C, normalized so inside <=> all w >= 0), plus masks used for the
branchless interval math.

Device (per core, faces sharded 8 ways):
  stage 1: layout [128 faces (partitions), 256 rows (free)].  Per edge the
    row-dependent term u = B*py + C and bound q = -u/A are tensor_scalar
    ops with per-partition scalars (the per-face coefficients).  Combine
    three edges into interval [LO, HI] in NDC, convert to x-index space,
    clamp to [-1, 257] (kills catastrophic cancellation from
    near-horizontal edges), form center c = (XLO+XHI)/2, half-width
    r = (XHI-XLO)/2, then rinv = 1/max(|r|, 1e-20) with empty intervals
    (r < 0) remapped to c = 1e6, rinv = 1 so they never cover a pixel.
  transpose: PE-transpose c, rinv to [128 rows, faces] layout.
  stage 2 (raster): for groups of G faces, z = |x - c| * rinv over the
    [128 rows, 256 x, G] broadcast layout; pixel inside iff min_g z <= 1.
    acc = min over groups; sil = (acc <= 1).
Partial silhouettes (OR over the core's faces) are max-reduced on host.
"""

import numpy as np

_B, _V, _F, _S = 2, 3889, 7774, 256
_NCORES = 8
_FPC = 1024            # faces per core, padded (8 subchunks of 128)
_NSUB = _FPC // 128
_G = 32                # faces per raster group
_NCOEF = 27            # 9 per edge * 3 edges
_BIG = 1.0e9

_F_PER_CORE_REAL = (_F + _NCORES - 1) // _NCORES  # 972


def _host_coeffs(vertices, faces, cams):
    """Per-core packed coefficient tensors CO[core][128, NSUB*NCOEF]."""
    v = vertices.astype(np.float32)
    cam = cams.astype(np.float32)

    # perspective projection, mirroring reference op-for-op (f32)
    f = cam[:, 0][:, None]                      # [B,1]
    z = v[:, :, 2] + np.float32(0.0)            # [B,V]
    image_size = cam[0, 1] * np.float32(2.0)
    cx = cam[:, 1][:, None]
    cy = cam[:, 2][:, None]
    px = f * v[:, :, 0] / z + cx
    py = f * v[:, :, 1] / z + cy
    ndc_x = px / image_size * np.float32(2.0) - np.float32(1.0)   # [B,V]
    ndc_y = py / image_size * np.float32(2.0) - np.float32(1.0)

    fi = faces.astype(np.int64)                 # [B,F,3]
    bidx = np.arange(_B)[:, None, None]
    tx = ndc_x[bidx, fi]                        # [B,F,3]
    ty = ndc_y[bidx, fi]

    v0x, v1x, v2x = tx[..., 0], tx[..., 1], tx[..., 2]
    v0y, v1y, v2y = ty[..., 0], ty[..., 1], ty[..., 2]

    area = (v1x - v0x) * (v2y - v0y) - (v1y - v0y) * (v2x - v0x)
    valid = np.abs(area) > np.float32(1e-12)
    # with w = cross(p - a, b - a), interior sign is -sign(area)
    s = np.where(area > 0, np.float32(-1.0), np.float32(1.0))
    s = np.where(valid, s, np.float32(0.0))

    # edges: w0=(v1,v2), w1=(v2,v0), w2=(v0,v1); w = A*px + B*py + C
    AA = np.empty((3, _B, _F), np.float32)
    BB = np.empty((3, _B, _F), np.float32)
    CC = np.empty((3, _B, _F), np.float32)
    for e, (ax, ay, bx, by) in enumerate(
        [(v1x, v1y, v2x, v2y), (v2x, v2y, v0x, v0y), (v0x, v0y, v1x, v1y)]
    ):
        dy = by - ay
        dx = bx - ax
        AA[e] = s * dy
        BB[e] = s * (-dx)
        CC[e] = s * (ay * dx - ax * dy)
    # invalid faces: force empty interval (A=B=0, C=-1 -> u=-1<0 with A==0)
    AA = np.where(valid[None], AA, np.float32(0.0))
    BB = np.where(valid[None], BB, np.float32(0.0))
    CC = np.where(valid[None], CC, np.float32(-1.0))

    m0 = AA == 0
    mpos = (AA > 0).astype(np.float32)
    mneg = (AA < 0).astype(np.float32)
    a_safe = np.where(m0, np.float32(1.0), AA)
    nra = np.float32(-1.0) / a_safe
    blo = np.float32(-_BIG) * (np.float32(1.0) - mpos)
    bhi = np.float32(_BIG) * (np.float32(1.0) - mneg)
    m2b = np.float32(2 * _BIG) * m0.astype(np.float32)
    m2bn = np.float32(-2 * _BIG) * m0.astype(np.float32)

    # coef order per edge: EB, EC, NRA, MPOS, BLO, M2B, MNEG, BHI, M2BN
    coef = np.stack([BB, CC, nra, mpos, blo, m2b, mneg, bhi, m2bn], axis=1)
    # coef: [3, 9, B, F] -> [B, F, 27]
    coef = coef.reshape(27, _B, _F).transpose(1, 2, 0)

    # pad faces to NCORES * FPC with empty-interval coeffs
    pad_coef = np.zeros((27,), np.float32)
    pad_coef[1] = -1.0          # EC = -1
    pad_coef[2] = -1.0          # NRA = -1/1
    pad_coef[4] = -_BIG         # BLO
    pad_coef[5] = 2 * _BIG      # M2B
    pad_coef[7] = _BIG          # BHI
    pad_coef[8] = -2 * _BIG     # M2BN
    full = np.empty((_B, _NCORES * _FPC, 27), np.float32)
    full[:] = pad_coef[None, None, :]

    for k in range(_NCORES):
        s0 = k * _F_PER_CORE_REAL
        s1 = min(_F, (k + 1) * _F_PER_CORE_REAL)
        full[:, k * _FPC : k * _FPC + (s1 - s0)] = coef[:, s0:s1]

    # pack per core: CO[b][p, sub*27 + idx] for face j = sub*128 + p
    cos = []
    for k in range(_NCORES):
        c = full[:, k * _FPC : (k + 1) * _FPC]          # [B, FPC, 27]
        c = c.reshape(_B, _NSUB, 128, 27)               # [B, sub, p, idx]
        c = c.transpose(2, 0, 1, 3).reshape(128, _B * _NSUB * 27)
        cos.append(np.ascontiguousarray(c))
    return cos


def _build_program():
    import concourse.bacc as bacc
    import concourse.mybir as mybir
    from concourse.tile import TileContext

    dt = mybir.dt.float32
    op = mybir.AluOpType
    nc = bacc.Bacc()

    co_ext = nc.declare_dram_parameter("co", [128, _B * _NSUB * _NCOEF], dt,
                                       isOutput=False)
    pyrep_ext = nc.declare_dram_parameter("pyrep", [128, _S], dt,
                                          isOutput=False)
    xrep_ext = nc.declare_dram_parameter("xrep", [128, _S], dt,
                                         isOutput=False)
    ident_ext = nc.declare_dram_parameter("ident", [128, 128], dt,
                                          isOutput=False)
    out_ext = nc.declare_dram_parameter("out", [_B, _S, _S], dt,
                                        isOutput=True)

    with TileContext(nc) as tc:
        with (
            tc.tile_pool(name="const", bufs=1) as cpool,
            tc.tile_pool(name="s1", bufs=1) as s1,
            tc.tile_pool(name="ct", bufs=1) as ctp,
            tc.tile_pool(name="ps", bufs=1, space="PSUM") as psp,
            tc.tile_pool(name="rast", bufs=1) as rp,
            tc.tile_pool(name="accp", bufs=1) as ap_,
        ):
            co0 = cpool.tile([128, _B * _NSUB * _NCOEF], dt, tag="co0")
            nc.sync.dma_start(out=co0[:, :], in_=co_ext[:, :])
            pyrep0 = cpool.tile([128, _S], dt, tag="pyrep0")
            nc.sync.dma_start(out=pyrep0[:, :], in_=pyrep_ext[:, :])
            xrep0 = cpool.tile([128, _S], dt, tag="xrep0")
            nc.sync.dma_start(out=xrep0[:, :], in_=xrep_ext[:, :])
            ident0 = cpool.tile([128, 128], dt, tag="ident0")
            nc.sync.dma_start(out=ident0[:, :], in_=ident_ext[:, :])
            # Re-materialize via VectorE so downstream DVE ops (which read
            # up to 3 of these at once) never exceed the per-instruction
            # sync-wait slot limit: same-engine deps need no semaphores.
            co = cpool.tile([128, _B * _NSUB * _NCOEF], dt, tag="co")
            nc.vector.tensor_copy(co[:, :], co0[:, :])
            ident = cpool.tile([128, 128], dt, tag="ident")
            nc.vector.tensor_copy(ident[:, :], ident0[:, :])
            pyrep = cpool.tile([128, _S], dt, tag="pyrep")
            nc.vector.tensor_copy(pyrep[:, :], pyrep0[:, :])
            xrep = cpool.tile([128, _S], dt, tag="xrep")
            nc.vector.tensor_copy(xrep[:, :], xrep0[:, :])

            for b in range(_B):
                ctps = [psp.tile([128, _FPC], dt, tag=f"ctps{blk}",
                                 name=f"ctps{blk}") for blk in range(2)]
                rtps = [psp.tile([128, _FPC], dt, tag=f"rtps{blk}",
                                 name=f"rtps{blk}") for blk in range(2)]
                for sub in range(_NSUB):
                    base = (b * _NSUB + sub) * _NCOEF

                    def col(idx):
                        return co[:, base + idx : base + idx + 1]

                    los, his = [], []
                    for e in range(3):
                        eb, ec, nra = col(9 * e), col(9 * e + 1), col(9 * e + 2)
                        mp, bl, m2b = col(9 * e + 3), col(9 * e + 4), col(9 * e + 5)
                        mn, bh, m2bn = col(9 * e + 6), col(9 * e + 7), col(9 * e + 8)

                        u = s1.tile([128, _S], dt, tag=f"u{e}")
                        nc.vector.tensor_scalar(u[:, :], pyrep[:, :], eb, ec,
                                                op.mult, op.add)
                        q = s1.tile([128, _S], dt, tag=f"q{e}")
                        nc.vector.tensor_scalar(q[:, :], u[:, :], nra, None,
                                                op.mult)
                        tlo = s1.tile([128, _S], dt, tag=f"tlo{e}")
                        nc.vector.tensor_scalar(tlo[:, :], u[:, :], 0.0, m2b,
                                                op.is_lt, op.mult)
                        thi = s1.tile([128, _S], dt, tag=f"thi{e}")
                        nc.vector.tensor_scalar(thi[:, :], u[:, :], 0.0, m2bn,
                                                op.is_lt, op.mult)
                        lo = s1.tile([128, _S], dt, tag=f"lo{e}")
                        nc.vector.tensor_scalar(lo[:, :], q[:, :], mp, bl,
                                                op.mult, op.add)
                        nc.vector.tensor_tensor(lo[:, :], lo[:, :], tlo[:, :],
                                                op.add)
                        hi = s1.tile([128, _S], dt, tag=f"hi{e}")
                        nc.vector.tensor_scalar(hi[:, :], q[:, :], mn, bh,
                                                op.mult, op.add)
                        nc.vector.tensor_tensor(hi[:, :], hi[:, :], thi[:, :],
                                                op.add)
                        los.append(lo)
                        his.append(hi)

                    LO = s1.tile([128, _S], dt, tag="LO")
                    nc.vector.tensor_tensor(LO[:, :], los[0][:, :], los[1][:, :],
                                            op.max)
                    nc.vector.tensor_tensor(LO[:, :], LO[:, :], los[2][:, :],
                                            op.max)
                    HI = s1.tile([128, _S], dt, tag="HI")
                    nc.vector.tensor_tensor(HI[:, :], his[0][:, :], his[1][:, :],
                                            op.min)
                    nc.vector.tensor_tensor(HI[:, :], HI[:, :], his[2][:, :],
                                            op.min)

                    # NDC -> x-index bounds, clamped to [-1, 257]
                    nc.vector.tensor_scalar(LO[:, :], LO[:, :], 128.0, 127.5,
                                            op.mult, op.add)
                    nc.vector.tensor_scalar(LO[:, :], LO[:, :], -1.0, 257.0,
                                            op.max, op.min)
                    nc.vector.tensor_scalar(HI[:, :], HI[:, :], 128.0, 127.5,
                                            op.mult, op.add)
                    nc.vector.tensor_scalar(HI[:, :], HI[:, :], -1.0, 257.0,
                                            op.max, op.min)

                    cc = s1.tile([128, _S], dt, tag="cc")
                    nc.vector.tensor_tensor(cc[:, :], LO[:, :], HI[:, :], op.add)
                    nc.vector.tensor_scalar(cc[:, :], cc[:, :], 0.5, None,
                                            op.mult)
                    rr = s1.tile([128, _S], dt, tag="rr")
                    nc.vector.tensor_tensor(rr[:, :], HI[:, :], LO[:, :],
                                            op.subtract)
                    nc.vector.tensor_scalar(rr[:, :], rr[:, :], 0.5, None,
                                            op.mult)

                    # empty mask m (r < 0), u1 = 1 - m
                    m = s1.tile([128, _S], dt, tag="m")
                    nc.vector.tensor_scalar(m[:, :], rr[:, :], 0.0, None,
                                            op.is_lt)
                    u1 = s1.tile([128, _S], dt, tag="u1")
                    nc.vector.tensor_scalar(u1[:, :], m[:, :], -1.0, 1.0,
                                            op.mult, op.add)
                    # c' = c*u1 + 1e6*m
                    cfix = s1.tile([128, _S], dt, tag="cfix")
                    nc.vector.tensor_scalar(cfix[:, :], m[:, :], 1.0e6, None,
                                            op.mult)
                    nc.vector.tensor_tensor(cc[:, :], cc[:, :], u1[:, :],
                                            op.mult)
                    nc.vector.tensor_tensor(cc[:, :], cc[:, :], cfix[:, :],
                                            op.add)
                    # rinv' = recip(max(|r|,1e-20))*u1 + m
                    rab = s1.tile([128, _S], dt, tag="rab")
                    nc.vector.tensor_scalar(rab[:, :], rr[:, :], 1.0e-20, None,
                                            op.max)
                    rinv = s1.tile([128, _S], dt, tag="rinv")
                    nc.vector.reciprocal(rinv[:, :], rab[:, :])
                    nc.vector.tensor_tensor(rinv[:, :], rinv[:, :], u1[:, :],
                                            op.mult)
                    nc.vector.tensor_tensor(rinv[:, :], rinv[:, :], m[:, :],
                                            op.add)

                    # transpose [faces, rows] -> [rows, faces]
                    for blk in range(2):
                        ys = slice(blk * 128, (blk + 1) * 128)
                        fs = slice(sub * 128, (sub + 1) * 128)
                        nc.tensor.transpose(ctps[blk][:, fs], cc[:, ys],
                                            ident[:, :])
                        nc.tensor.transpose(rtps[blk][:, fs], rinv[:, ys],
                                            ident[:, :])

                for blk in range(2):
                    ct = ctp.tile([128, _FPC], dt, tag=f"ct{blk}")
                    nc.scalar.copy(ct[:, :], ctps[blk][:, :])
                    rt = ctp.tile([128, _FPC], dt, tag=f"rt{blk}")
                    nc.scalar.copy(rt[:, :], rtps[blk][:, :])

                    acc = ap_.tile([128, _S], dt, tag="acc")
                    nc.vector.memset(acc[:, :], _BIG)
                    for g in range(_FPC // _G):
                        gsl = slice(g * _G, (g + 1) * _G)
                        x3 = xrep[:, :].unsqueeze(2).broadcast_to(
                            (128, _S, _G))
                        c3 = ct[:, gsl].unsqueeze(1).broadcast_to(
                            (128, _S, _G))
                        r3 = rt[:, gsl].unsqueeze(1).broadcast_to(
                            (128, _S, _G))
                        d = rp.tile([128, _S * _G], dt, tag="d")
                        d3 = d[:, :].rearrange("p (x g) -> p x g", g=_G)
                        nc.vector.tensor_tensor(d3, x3, c3, op.subtract)
                        z = rp.tile([128, _S * _G], dt, tag="z")
                        z3 = z[:, :].rearrange("p (x g) -> p x g", g=_G)
                        nc.vector.tensor_tensor(z3, d3, r3, op.mult)
                        red = rp.tile([128, _S], dt, tag="red")
                        nc.vector.tensor_reduce(red[:, :], z3,
                                                mybir.AxisListType.X, op.min,
                                                apply_absolute_value=True)
                        nc.vector.tensor_tensor(acc[:, :], acc[:, :],
                                                red[:, :], op.min)

                    sil = ap_.tile([128, _S], dt, tag="sil")
                    nc.vector.tensor_scalar(sil[:, :], acc[:, :], 1.0, None,
                                            op.is_le)
                    nc.sync.dma_start(
                        out=out_ext[b, blk * 128 : (blk + 1) * 128, :],
                        in_=sil[:, :])
    nc.compile()
    return nc


_PROGRAM = None


def kernel(vertices, faces, cams):
    global _PROGRAM
    from concourse.bass_utils import run_bass_kernel_spmd

    cos = _host_coeffs(vertices, faces, cams)

    ys = (np.float32(2.0) * np.arange(_S, dtype=np.float32) + np.float32(1.0)) \
        / np.float32(_S) - np.float32(1.0)
    pyrep = np.broadcast_to(ys[None, :], (128, _S)).copy()
    xrep = np.broadcast_to(np.arange(_S, dtype=np.float32)[None, :],
                           (128, _S)).copy()
    ident = np.eye(128, dtype=np.float32)

    if _PROGRAM is None:
        _PROGRAM = _build_program()

    in_maps = [
        {"co": cos[k], "pyrep": pyrep, "xrep": xrep, "ident": ident}
        for k in range(_NCORES)
    ]
    res = run_bass_kernel_spmd(_PROGRAM, in_maps, list(range(_NCORES)))
    outs = [res.results[k]["out"] for k in range(_NCORES)]
    sil = outs[0]
    for k in range(1, _NCORES):
        sil = np.maximum(sil, outs[k])
    return sil.astype(np.float32)
